# revision 18
# baseline (speedup 1.0000x reference)
"""Trainium2 Bass kernel for nn_MultiHeadAttention_910533067646.

Self-contained: builds the Bass module, shards the full inputs across the
8 NeuronCores (data-parallel over batch x tensor-parallel over heads), runs
via PJRT, and reassembles the full output.

The reference uses one shared projection p = x @ Wv.T + bv for q=k=v, so the
per-head attention matrix E = exp(p p^T/8) is SYMMETRIC. This kernel
computes only the upper-triangle 128x128 tiles of E (exp on ~53% of the
matrix) and materializes the lower tiles with DMA XBAR transposes (free wrt
the compute engines). The attention row-sums come for free from a ones
column appended to the stationary of the attn@v matmul (PSUM row 64).
The bugged head-major reshape in the reference maps each head's output to a
disjoint 128-row block of the final output, so device (b, hg) computes
output rows [1024*hg, 1024*hg+1024) of batch b with no cross-device
reduction.
"""
import numpy as np

from contextlib import ExitStack

import concourse.bass as bass
import concourse.mybir as mybir
import concourse.tile as tile
from concourse.masks import make_identity

FP = mybir.dt.float32
FPR = mybir.dt.float32r
FP16 = mybir.dt.float16
BF16 = mybir.dt.bfloat16
Exp = mybir.ActivationFunctionType.Exp
ADD = mybir.AluOpType.add
MULT = mybir.AluOpType.mult


def _build_mha_nc(S=2048, D=1024, HL=8, dk=64, phases="ABMCNF", MM=FPR,
                  loop_bcnf=1, dbg=False):
    EL = HL * dk            # local width of the value projection (512)
    KK = D // 128           # contraction k-tiles (8)
    NG = HL // 2            # head pairs (4)
    NB = S // 128           # 128-row strips of the sequence (16)
    NTRI = NB * (NB + 1) // 2   # upper-triangle tiles per head (136)
    base = [0] * NB         # strip i holds tiles (i, j>=i) at base[i]..
    for i in range(1, NB):
        base[i] = base[i - 1] + (NB - (i - 1))
    TP = NB // 2            # t-pair steps of the output projection (8)
    assert EL == 512 and S == 2048 and dk == 64 and HL == 8

    nc = bass.Bass("TRN2")
    xT_d = nc.dram_tensor("xT", [D, S], FP16, kind="ExternalInput")
    wvT_d = nc.dram_tensor("wvT", [D, EL], FP16, kind="ExternalInput")
    woT_d = nc.dram_tensor("woT", [D, D], BF16, kind="ExternalInput")
    bv_d = nc.dram_tensor("bv", [1, EL], FP, kind="ExternalInput")
    bo_d = nc.dram_tensor("bo", [1, D], FP, kind="ExternalInput")
    out_d = nc.dram_tensor("out", [128 * HL, D], FP, kind="ExternalOutput")

    with ExitStack() as stk:
        tc = stk.enter_context(tile.TileContext(nc))
        const = stk.enter_context(tc.tile_pool(name="const", bufs=1))
        ppool = stk.enter_context(tc.tile_pool(name="ppool", bufs=1))
        epool = stk.enter_context(tc.tile_pool(name="epool", bufs=8))
        ps_m = stk.enter_context(tc.tile_pool(name="ps_m", bufs=2, space="PSUM"))

        ones32 = const.tile([1, 512], FP, name="ones32")
        ones_r = const.tile([1, 64], MM, name="ones_r")
        ones_h = const.tile([1, 512], FP16, name="ones_h")
        ones_bf = const.tile([1, 128], BF16, name="ones_bf")
        bv_sb = const.tile([1, EL], FP, name="bv_sb")
        bv_h = const.tile([1, EL], FP16, name="bv_h")
        bo_sb = const.tile([1, D], FP, name="bo_sb")
        bo_bf = const.tile([1, D], BF16, name="bo_bf")
        ident = const.tile([128, 128], FP, name="ident")
        ident16 = const.tile([128, 128], FP16, name="ident16")
        bias_sb = const.tile([128, HL], FP, name="bias_sb")
        nc.sync.dma_start(bv_sb[:], bv_d[:])
        nc.sync.dma_start(bo_sb[:], bo_d[:])
        nc.gpsimd.memset(ones32[:], 1.0)
        nc.gpsimd.memset(ones_h[:], 1.0)
        nc.vector.tensor_copy(ones_bf[:], ones32[0:1, 0:128])
        nc.vector.tensor_copy(ones_r[:], ones32[0:1, 0:64])
        nc.vector.tensor_copy(bv_h[:], bv_sb[:])
        nc.vector.tensor_copy(bo_bf[:], bo_sb[:])
        make_identity(nc, ident[:])
        make_identity(nc, ident16[:])

        pT_sb = ppool.tile([128, NG, S], FP16, name="pT_sb")
        p65 = ppool.tile([128, NB, HL, dk + 1], FP16, name="p65")
        # ones column for the attn@v row-sum trick
        nc.gpsimd.memset(p65[:, :, :, dk:dk + 1], 1.0)

        xt_ctx = tc.tile_pool(name="xtpool", bufs=1)
        xtpool = xt_ctx.__enter__()
        wvT_sb = xtpool.tile([128, KK, EL], FP16, name="wvT_sb")
        xT_sb = xtpool.tile([128, KK, S], FP16, name="xT_sb")
        nc.sync.dma_start(wvT_sb[:],
                          wvT_d[:].rearrange("(kk p) e -> p kk e", p=128))
        for kk in range(KK):
            nc.sync.dma_start(xT_sb[:, kk, :], xT_d[128 * kk:128 * (kk + 1), :])

        # ---- phase A: projection p (per strip) + pT (transposed) ----
        def emit_p(j):
            ps = ps_m.tile([128, EL], FP, name="ps_p", tag="sc")
            for kk in range(KK):
                nc.tensor.matmul(ps[:], xT_sb[:, kk, 128 * j:128 * (j + 1)],
                                 wvT_sb[:, kk, :], start=(kk == 0), stop=False)
            nc.tensor.matmul(ps[:], ones_h[0:1, 0:128], bv_h[0:1, :],
                             start=False, stop=True)
            nc.vector.tensor_copy(
                p65[:, j, :, 0:dk], ps[:].rearrange("p (a b) -> p a b", a=HL))

        def emit_pT0(ns):   # pair 0 direct (B of head 0 starts early)
            ps = ps_m.tile([128, 512], FP, name="ps_pt", tag="sc")
            for kk in range(KK):
                nc.tensor.matmul(ps[:], wvT_sb[:, kk, 0:128],
                                 xT_sb[:, kk, 512 * ns:512 * (ns + 1)],
                                 start=(kk == 0), stop=False)
            nc.tensor.matmul(ps[:], bv_h[0:1, 0:128], ones_h[0:1, 0:512],
                             start=False, stop=True)
            nc.vector.tensor_copy(pT_sb[:, 0, 512 * ns:512 * (ns + 1)], ps[:])

        def emit_T(g, j):   # pairs 1..3 via PE transpose of p
            ps = ps_m.tile([128, 128], FP16, name="ps_t0", tag="sc")
            for a2 in range(2):
                nc.tensor.transpose(ps[64 * a2:64 * (a2 + 1), :],
                                    p65[:, j, 2 * g + a2, 0:dk], ident16[:],
                                    tile_position=(0, 64 * a2))
            nc.vector.tensor_copy(pT_sb[:, g, 128 * j:128 * (j + 1)], ps[:])

        for ns in range(S // 512):
            emit_pT0(ns)
        for j in range(NB):
            emit_p(j)
            for g in range(1, NG):
                emit_T(g, j)

        # post pools reuse the xT/wvT address space
        xt_ctx.__exit__(None, None, None)
        wpool = stk.enter_context(tc.tile_pool(name="wpool", bufs=1))
        npool = stk.enter_context(tc.tile_pool(name="npool", bufs=2))
        woT_sb = wpool.tile([128, TP, D], BF16, name="woT_sb")
        Edir = wpool.tile([128, NTRI, 128], FP16, name="Edir")
        Mbuf = wpool.tile([128, NB, NB - 1, 128], FP16, name="Mbuf")
        nc.scalar.dma_start(woT_sb[:],
                            woT_d[:].rearrange("(t p) e -> p t e", p=128))
        Mflat = Mbuf[:].rearrange("p a b c -> p (a b) c")

        if "B" not in phases:
            return nc

        loop_cm = None
        if loop_bcnf > 1:
            loop_cm = tc.For_i(0, loop_bcnf, 1)
            loop_cm.__enter__()

        # ---- exp bias: bias[al] = 10 - max_q |p_q,al|^2 / 8 per local head.
        # Cauchy-Schwarz: s[q,k] <= max |p|^2, so exp(s/8 + bias) <= e^10
        # keeps fp16 E from overflowing.
        def emit_shift_all():
            dg = epool.tile([128, HL, NB], FP, name="dg", tag="dg", bufs=1)
            for j in range(NB):
                scr = epool.tile([128, HL, dk], FP, name="scr", tag="scr",
                                 bufs=2)
                nc.vector.tensor_tensor(scr[:], p65[:, j, :, 0:dk],
                                        p65[:, j, :, 0:dk], MULT)
                nc.vector.reduce_sum(dg[:, :, j], scr[:],
                                     axis=mybir.AxisListType.X)
            cm8 = epool.tile([128, HL], FP, name="cm8", tag="cm8", bufs=1)
            nc.vector.reduce_max(cm8[:], dg[:], axis=mybir.AxisListType.X)
            ps_t1 = ps_m.tile([HL, 128], FP, name="ps_t1", tag="sc")
            nc.tensor.transpose(ps_t1[:], cm8[:], ident[:])
            cmT = epool.tile([HL, 128], FP, name="cmT", tag="cmT", bufs=1)
            nc.vector.tensor_copy(cmT[:], ps_t1[:])
            m8 = epool.tile([HL, 1], FP, name="m8", tag="m8", bufs=1)
            nc.vector.reduce_max(m8[:], cmT[:], axis=mybir.AxisListType.X)
            ps_t2 = ps_m.tile([1, HL], FP, name="ps_t2", tag="sc")
            nc.tensor.transpose(ps_t2[:], m8[:], ident[0:HL, 0:HL])
            m8T = epool.tile([1, HL], BF16, name="m8T", tag="m8T", bufs=1)
            nc.vector.tensor_copy(m8T[:], ps_t2[:])
            for al in range(HL):
                ps_b = ps_m.tile([128, 1], FP, name="ps_b", tag="sc")
                nc.tensor.matmul(ps_b[:], ones_bf[0:1, :], m8T[0:1, al:al + 1])
                nc.vector.tensor_scalar(bias_sb[:, al:al + 1], ps_b[:],
                                        -0.125, 10.0, MULT, ADD)

        emit_shift_all()

        # ---- per-head B (triangle scores+exp) / mirror / C / norm / F ----
        pending_nf = [None]

        def emit_head(al):
            g, lo = al // 2, 64 * (al % 2)
            cpart_box = [None]   # allocated lazily at C(0), after pending nf

            def emit_B(i):
                # direct strip i: score tiles (i, j>=i), then exp into Edir
                W = (NB - i) * 128
                for s0 in range(0, W, 1024):
                    w = min(1024, W - s0)
                    sc = ps_m.tile([128, 1024], FP, name="sc", tag="sc")
                    for ns in range(0, w, 512):
                        nw = min(512, w - ns)
                        nc.tensor.matmul(
                            sc[:, ns:ns + nw],
                            pT_sb[lo:lo + 64, g, 128 * i:128 * (i + 1)],
                            pT_sb[lo:lo + 64, g,
                                  128 * i + s0 + ns:128 * i + s0 + ns + nw],
                            tile_position=(lo, 0))
                    k0 = base[i] + s0 // 128
                    nc.scalar.activation(
                        Edir[:, k0:k0 + w // 128, :].rearrange("p a b -> p (a b)"),
                        sc[:, 0:w], Exp, scale=1.0 / 8.0,
                        bias=bias_sb[:, al:al + 1])

            def emit_mirror(i):
                # lower tiles (i, j>i) -> Mbuf slot i via DMA XBAR transpose
                n = NB - 1 - i
                if n == 0:
                    return
                nc.sync.dma_start_transpose(
                    Mbuf[:, i, 0:n, :],
                    Edir[:, base[i] + 1:base[i] + 1 + n, :]
                    .rearrange("p a b -> p (a b)"))

            def emit_C(i):
                # cpart[:, s] += p65_i^T @ E[strip i, all s]; row 64 = sums
                if cpart_box[0] is None:
                    cpart_box[0] = ps_m.tile([dk + 1, S], FP, name="cpart",
                                             tag="cp", bufs=1)
                cpart = cpart_box[0]
                lhsT = p65[:, i, al, :]
                for b in range(4):
                    mi = max(0, min(4 * b + 4, i) - 4 * b)
                    di = 4 - mi
                    if mi > 0:
                        st = 14 * 4 * b + i - 1
                        nc.tensor.matmul(
                            cpart[:, 512 * b:512 * b + 128 * mi],
                            lhsT,
                            Mflat[:, st:st + 14 * mi:14, :],
                            start=False, stop=(i == NB - 1 and di == 0),
                            skip_group_check=True)
                    if di > 0:
                        t0 = max(4 * b, i)
                        nc.tensor.matmul(
                            cpart[:, 512 * b + 128 * mi:512 * (b + 1)],
                            lhsT,
                            Edir[:, base[i] + t0 - i:base[i] + t0 - i + di, :],
                            start=(i == 0), stop=(i == NB - 1),
                            skip_group_check=True)

            LAG = 2
            for i in range(NB):
                emit_B(i)
                if "M" in phases:
                    emit_mirror(i)
                if i == 2 and pending_nf[0] is not None:
                    pending_nf[0]()
                    pending_nf[0] = None
                if "C" in phases and i - LAG >= 0:
                    emit_C(i - LAG)
            if "C" in phases:
                for i in range(NB - LAG, NB):
                    emit_C(i)

            def nf():
                # softmax normalization + output projection for head al
                cpart = cpart_box[0]
                recip = npool.tile([1, S], MM, name="recip", tag="recip",
                                   bufs=1)
                with nc.allow_low_precision(reason="recip feeds fp32r bc"):
                    nc.vector.reciprocal(recip[:], cpart[dk:dk + 1, :])
                normD = npool.tile([128, S], BF16, name="normD", tag="normD")
                if "1" in phases:
                    return
                for hh in range(2):
                    bc = ps_m.tile([128, 1024], FP, name="bc", tag="sc")
                    for ns in range(2):
                        nc.tensor.matmul(
                            bc[0:dk, 512 * ns:512 * (ns + 1)],
                            ones_r[0:1, :],
                            recip[0:1, 1024 * hh + 512 * ns:
                                  1024 * hh + 512 * (ns + 1)])
                    if "4" in phases:
                        continue
                    bc_sb = npool.tile([dk, 1024], FP, name="bc_sb",
                                       tag="bc_sb")
                    nc.vector.tensor_copy(bc_sb[:], bc[0:dk, :])
                    if "5" in phases:
                        continue
                    nc.vector.tensor_tensor(
                        normD[0:dk, 1024 * hh:1024 * (hh + 1)],
                        cpart[0:dk, 1024 * hh:1024 * (hh + 1)], bc_sb[:], MULT)
                if "2" in phases:
                    return
                # upper half = lower shifted left by one position (odd c's)
                nc.sync.dma_start(normD[64:128, 0:S - 1], normD[0:64, 1:S])
                if "3" in phases:
                    return

                fps = ps_m.tile([128, 1024], FP, name="fps", tag="sc")
                for ns in range(2):
                    nc.tensor.matmul(fps[:, 512 * ns:512 * (ns + 1)],
                                     ones_bf[0:1, 0:128],
                                     bo_bf[0:1, 512 * ns:512 * (ns + 1)],
                                     start=True, stop=False,
                                     skip_group_check=True)
                for tp in range(TP):
                    for ns in range(2):
                        nc.tensor.matmul(
                            fps[:, 512 * ns:512 * (ns + 1)],
                            normD[:, 2 * tp::NB],
                            woT_sb[:, tp, 512 * ns:512 * (ns + 1)],
                            start=False, stop=(tp == TP - 1),
                            skip_group_check=True)
                fsb = npool.tile([128, D], FP, name="fsb", tag="fsb")
                nc.vector.tensor_copy(fsb[:], fps[:])
                nc.sync.dma_start(out_d[128 * al:128 * (al + 1), :], fsb[:])

            if "N" in phases:
                pending_nf[0] = nf

        for al in range(HL):
            emit_head(al)
        if pending_nf[0] is not None:
            pending_nf[0]()
            pending_nf[0] = None

        if loop_cm is not None:
            loop_cm.__exit__(None, None, None)

    return nc


def _split_excess_waits(nc, max_waits=1):
    """This toolchain's walrus accepts only one sync-wait per instruction;
    hoist extra waits onto NoOps inserted just before."""
    fn = nc.m.functions[0]
    n_new = 0
    for blk in fn.blocks:
        new_insts = []
        for inst in blk.instructions:
            si = getattr(inst, 'sync_info', None)
            if si is not None and si.on_wait is not None \
                    and len(si.on_wait) > max_waits:
                waits = list(si.on_wait)
                while len(waits) > max_waits:
                    chunk, waits = waits[:max_waits], waits[max_waits:]
                    n_new += 1
                    new_insts.append(mybir.InstNoOp(
                        name=f"I-waitsplit-{n_new}", engine=inst.engine,
                        ins=[], outs=[],
                        sync_info=mybir.SyncInfo(on_wait=chunk, on_update=[]),
                        bass_nofuse=True))
                si.on_wait = waits
            new_insts.append(inst)
        blk.instructions = new_insts
    return n_new


class _PjrtRunner:
    def __init__(self, nc, n_cores):
        import jax
        from jax.sharding import Mesh, PartitionSpec
        from jax.experimental.shard_map import shard_map
        from concourse.bass2jax import (_bass_exec_p, partition_id_tensor,
                                        install_neuronx_cc_hook)
        install_neuronx_cc_hook()
        self.jax = jax
        self.n_cores = n_cores
        pname = nc.partition_id_tensor.name if nc.partition_id_tensor else None
        in_names, out_names, out_avals, zero_outs = [], [], [], []
        for alloc in nc.m.functions[0].allocations:
            if not isinstance(alloc, mybir.MemoryLocationSet):
                continue
            name = alloc.memorylocations[0].name
            if alloc.kind == "ExternalInput":
                if name != pname:
                    in_names.append(name)
            elif alloc.kind == "ExternalOutput":
                shape = tuple(alloc.tensor_shape)
                dtype = mybir.dt.np(alloc.dtype)
                out_names.append(name)
                out_avals.append(jax.core.ShapedArray(shape, dtype))
                zero_outs.append(np.zeros(shape, dtype))
        self.in_names, self.out_names = in_names, out_names
        self.out_avals, self.zero_outs = out_avals, zero_outs
        n_params, n_outs = len(in_names), len(out_avals)
        self.n_params = n_params
        all_in = in_names + out_names + ([pname] if pname else [])

        def _body(*args):
            operands = list(args)
            if pname is not None:
                operands.append(partition_id_tensor())
            return tuple(_bass_exec_p.bind(
                *operands, out_avals=tuple(out_avals), in_names=tuple(all_in),
                out_names=tuple(out_names), lowering_input_output_aliases=(),
                sim_require_finite=True, sim_require_nnan=True, nc=nc))

        devices = jax.devices()[:n_cores]
        self.mesh = Mesh(np.asarray(devices), ("core",))
        in_specs = (PartitionSpec("core"),) * (n_params + n_outs)
        out_specs = (PartitionSpec("core"),) * n_outs
        self.fn = jax.jit(
            shard_map(_body, mesh=self.mesh, in_specs=in_specs,
                      out_specs=out_specs, check_rep=False), keep_unused=True)
        self.PartitionSpec = PartitionSpec

    def run(self, in_maps):
        jax = self.jax
        per_core = [[np.asarray(m[n]) for n in self.in_names] for m in in_maps]
        concat_in = [np.concatenate([per_core[c][i] for c in range(self.n_cores)],
                                    axis=0) for i in range(self.n_params)]
        concat_zeros = [np.zeros((self.n_cores * z.shape[0], *z.shape[1:]),
                                 z.dtype) for z in self.zero_outs]
        sharding = jax.sharding.NamedSharding(self.mesh, self.PartitionSpec("core"))
        dev_in = [jax.device_put(a, sharding) for a in concat_in + concat_zeros]
        outs = self.fn(*dev_in)
        jax.block_until_ready(outs)
        return [
            {n: np.asarray(outs[i]).reshape(self.n_cores,
                                            *self.out_avals[i].shape)[c]
             for i, n in enumerate(self.out_names)}
            for c in range(self.n_cores)
        ]


_CACHE = {}

B_, S_, D_, H_, DK_ = 4, 2048, 1024, 16, 64
HL_ = H_ // 2          # heads per device
EL_ = HL_ * DK_        # value-projection width per device


def _make_in_maps(x, Wv, bv, Wo, bo):
    import ml_dtypes
    woT = np.ascontiguousarray(Wo.T).astype(ml_dtypes.bfloat16)
    in_maps = []
    for dev in range(8):
        b, hg = dev // 2, dev % 2
        in_maps.append({
            "xT": np.ascontiguousarray(x[b].T).astype(np.float16),
            "wvT": np.ascontiguousarray(
                Wv[EL_ * hg:EL_ * (hg + 1), :].T).astype(np.float16),
            "woT": woT,
            "bv": np.ascontiguousarray(
                bv[EL_ * hg:EL_ * (hg + 1)]).reshape(1, -1).astype(np.float32),
            "bo": np.ascontiguousarray(bo).reshape(1, -1).astype(np.float32),
        })
    return in_maps


def kernel(x, Wv, bv, Wo, bo):
    x, Wv, bv = np.asarray(x), np.asarray(Wv), np.asarray(bv)
    Wo, bo = np.asarray(Wo), np.asarray(bo)
    if "r" not in _CACHE:
        nc = _build_mha_nc(S=S_, D=D_, HL=HL_, dk=DK_)
        _split_excess_waits(nc)
        _CACHE["r"] = _PjrtRunner(nc, 8)
    r = _CACHE["r"]
    res = r.run(_make_in_maps(x, Wv, bv, Wo, bo))
    out = np.zeros((B_, S_, D_), np.float32)
    for dev in range(8):
        b, hg = dev // 2, dev % 2
        out[b, 1024 * hg:1024 * (hg + 1), :] = res[dev]["out"]
    return out


# revision 35
# speedup vs baseline: 1.0157x; 1.0157x over previous
"""Trainium2 Bass kernel for nn_MultiHeadAttention_910533067646.

Self-contained: builds the Bass module, shards the full inputs across the
8 NeuronCores (data-parallel over batch x tensor-parallel over heads), runs
via PJRT, and reassembles the full output.

The reference uses one shared projection p = x @ Wv.T + bv for q=k=v, so the
per-head attention matrix E = exp(p p^T/8) is SYMMETRIC. This kernel
computes only the upper-triangle 128x128 tiles of E (exp on ~53% of the
matrix) and materializes the lower tiles with DMA XBAR transposes (free wrt
the compute engines). The attention row-sums come for free from a ones
column appended to the stationary of the attn@v matmul (PSUM row 64).
The bugged head-major reshape in the reference maps each head's output to a
disjoint 128-row block of the final output, so device (b, hg) computes
output rows [1024*hg, 1024*hg+1024) of batch b with no cross-device
reduction.
"""
import numpy as np

from contextlib import ExitStack

import concourse.bass as bass
import concourse.mybir as mybir
import concourse.tile as tile
from concourse.masks import make_identity

FP = mybir.dt.float32
FPR = mybir.dt.float32r
FP16 = mybir.dt.float16
BF16 = mybir.dt.bfloat16
Exp = mybir.ActivationFunctionType.Exp
ADD = mybir.AluOpType.add
MULT = mybir.AluOpType.mult


def _build_mha_nc(S=2048, D=1024, HL=8, dk=64, phases="ABMCNF", MM=FPR,
                  loop_bcnf=1, dbg=False):
    EL = HL * dk            # local width of the value projection (512)
    KK = D // 128           # contraction k-tiles (8)
    NG = HL // 2            # head pairs (4)
    NB = S // 128           # 128-row strips of the sequence (16)
    NTRI = NB * (NB + 1) // 2   # upper-triangle tiles per head (136)
    base = [0] * NB         # strip i holds tiles (i, j>=i) at base[i]..
    for i in range(1, NB):
        base[i] = base[i - 1] + (NB - (i - 1))
    TP = NB // 2            # t-pair steps of the output projection (8)
    assert EL == 512 and S == 2048 and dk == 64 and HL == 8

    nc = bass.Bass("TRN2")
    xT_d = nc.dram_tensor("xT", [D, S], FP16, kind="ExternalInput")
    wvT_d = nc.dram_tensor("wvT", [D, EL], FP16, kind="ExternalInput")
    woT_d = nc.dram_tensor("woT", [D, D], BF16, kind="ExternalInput")
    bv_d = nc.dram_tensor("bv", [1, EL], FP, kind="ExternalInput")
    bo_d = nc.dram_tensor("bo", [1, D], FP, kind="ExternalInput")
    out_d = nc.dram_tensor("out", [128 * HL, D], FP, kind="ExternalOutput")

    with ExitStack() as stk:
        tc = stk.enter_context(tile.TileContext(nc))
        const = stk.enter_context(tc.tile_pool(name="const", bufs=1))
        ppool = stk.enter_context(tc.tile_pool(name="ppool", bufs=1))
        epool = stk.enter_context(tc.tile_pool(name="epool", bufs=8))
        ps_m = stk.enter_context(tc.tile_pool(name="ps_m", bufs=2, space="PSUM"))

        ones32 = const.tile([1, 512], FP, name="ones32")
        ones_r = const.tile([1, 64], MM, name="ones_r")
        ones_h = const.tile([1, 512], FP16, name="ones_h")
        ones_bf = const.tile([1, 128], BF16, name="ones_bf")
        bv_sb = const.tile([1, EL], FP, name="bv_sb")
        bv_h = const.tile([1, EL], FP16, name="bv_h")
        bo_sb = const.tile([1, D], FP, name="bo_sb")
        bo_bf = const.tile([1, D], BF16, name="bo_bf")
        ident = const.tile([128, 128], FP, name="ident")
        ident16 = const.tile([128, 128], FP16, name="ident16")
        bias_sb = const.tile([128, HL], FP, name="bias_sb")
        nc.sync.dma_start(bv_sb[:], bv_d[:])
        nc.sync.dma_start(bo_sb[:], bo_d[:])
        nc.gpsimd.memset(ones32[:], 1.0)
        nc.gpsimd.memset(ones_h[:], 1.0)
        nc.vector.tensor_copy(ones_bf[:], ones32[0:1, 0:128])
        nc.vector.tensor_copy(ones_r[:], ones32[0:1, 0:64])
        nc.vector.tensor_copy(bv_h[:], bv_sb[:])
        nc.vector.tensor_copy(bo_bf[:], bo_sb[:])
        make_identity(nc, ident[:])
        make_identity(nc, ident16[:])

        pT_sb = ppool.tile([128, NG, S], FP16, name="pT_sb")
        p65 = ppool.tile([128, NB, HL, dk + 1], FP16, name="p65")
        # ones column for the attn@v row-sum trick
        nc.gpsimd.memset(p65[:, :, :, dk:dk + 1], 1.0)

        xt_ctx = tc.tile_pool(name="xtpool", bufs=1)
        xtpool = xt_ctx.__enter__()
        wvT_sb = xtpool.tile([128, KK, EL], FP16, name="wvT_sb")
        xT_sb = xtpool.tile([128, KK, S], FP16, name="xT_sb")
        nc.sync.dma_start(wvT_sb[:],
                          wvT_d[:].rearrange("(kk p) e -> p kk e", p=128))
        for kk in range(KK):
            eng = nc.sync if kk % 2 == 0 else nc.scalar
            eng.dma_start(xT_sb[:, kk, :], xT_d[128 * kk:128 * (kk + 1), :])

        # ---- phase A: projection p (per strip) + pT (transposed) ----
        def emit_p(j):
            ps = ps_m.tile([128, EL], FP, name="ps_p", tag="sc")
            for kk in range(KK):
                nc.tensor.matmul(ps[:], xT_sb[:, kk, 128 * j:128 * (j + 1)],
                                 wvT_sb[:, kk, :], start=(kk == 0), stop=False)
            nc.tensor.matmul(ps[:], ones_h[0:1, 0:128], bv_h[0:1, :],
                             start=False, stop=True)
            nc.vector.tensor_copy(
                p65[:, j, :, 0:dk], ps[:].rearrange("p (a b) -> p a b", a=HL))

        def emit_pT0(ns):   # pair 0 direct (B of head 0 starts early)
            ps = ps_m.tile([128, 512], FP, name="ps_pt", tag="sc")
            for kk in range(KK):
                nc.tensor.matmul(ps[:], wvT_sb[:, kk, 0:128],
                                 xT_sb[:, kk, 512 * ns:512 * (ns + 1)],
                                 start=(kk == 0), stop=False)
            nc.tensor.matmul(ps[:], bv_h[0:1, 0:128], ones_h[0:1, 0:512],
                             start=False, stop=True)
            nc.vector.tensor_copy(pT_sb[:, 0, 512 * ns:512 * (ns + 1)], ps[:])

        def emit_T(g, j):   # pairs 1..3 via PE transpose of p
            ps = ps_m.tile([128, 128], FP16, name="ps_t0", tag="sc")
            for a2 in range(2):
                nc.tensor.transpose(ps[64 * a2:64 * (a2 + 1), :],
                                    p65[:, j, 2 * g + a2, 0:dk], ident16[:],
                                    tile_position=(0, 64 * a2))
            nc.vector.tensor_copy(pT_sb[:, g, 128 * j:128 * (j + 1)], ps[:])

        for ns in range(S // 512):
            emit_pT0(ns)
        for j in range(NB):
            emit_p(j)
            for g in range(1, NG):
                emit_T(g, j)

        # post pools reuse the xT/wvT address space
        xt_ctx.__exit__(None, None, None)
        wpool = stk.enter_context(tc.tile_pool(name="wpool", bufs=1))
        npool = stk.enter_context(tc.tile_pool(name="npool", bufs=2))
        woT_sb = wpool.tile([128, TP, D], BF16, name="woT_sb")
        Edir = wpool.tile([128, NTRI, 128], FP16, name="Edir")
        Mbuf = wpool.tile([128, NB, NB - 1, 128], FP16, name="Mbuf")
        # slots 0/1 are double-buffered by head parity: their mirrors can
        # then issue 4 strips ahead of the consuming C-step without
        # colliding with the previous head's deferred tail C reads
        MbufP = wpool.tile([128, 2, 2, NB - 1, 128], FP16, name="MbufP")
        nc.scalar.dma_start(woT_sb[:],
                            woT_d[:].rearrange("(t p) e -> p t e", p=128))
        Mflat = Mbuf[:].rearrange("p a b c -> p (a b) c")
        MPflat = MbufP[:].rearrange("p a b c d -> p (a b c) d")

        if "B" not in phases:
            return nc

        loop_cm = None
        if loop_bcnf > 1:
            loop_cm = tc.For_i(0, loop_bcnf, 1)
            loop_cm.__enter__()

        # ---- exp bias: bias[al] = 10 - max_q |p_q,al|^2 / 8 per local head.
        # Cauchy-Schwarz: s[q,k] <= max |p|^2, so exp(s/8 + bias) <= e^10
        # keeps fp16 E from overflowing.
        def emit_shift_all():
            dg = epool.tile([128, HL, NB], FP, name="dg", tag="dg", bufs=1)
            for j in range(NB):
                scr = epool.tile([128, HL, dk], FP, name="scr", tag="scr",
                                 bufs=2)
                nc.vector.tensor_tensor(scr[:], p65[:, j, :, 0:dk],
                                        p65[:, j, :, 0:dk], MULT)
                nc.vector.reduce_sum(dg[:, :, j], scr[:],
                                     axis=mybir.AxisListType.X)
            cm8 = epool.tile([128, HL], FP, name="cm8", tag="cm8", bufs=1)
            nc.vector.reduce_max(cm8[:], dg[:], axis=mybir.AxisListType.X)
            ps_t1 = ps_m.tile([HL, 128], FP, name="ps_t1", tag="sc")
            nc.tensor.transpose(ps_t1[:], cm8[:], ident[:])
            cmT = epool.tile([HL, 128], FP, name="cmT", tag="cmT", bufs=1)
            nc.vector.tensor_copy(cmT[:], ps_t1[:])
            m8 = epool.tile([HL, 1], FP, name="m8", tag="m8", bufs=1)
            nc.vector.reduce_max(m8[:], cmT[:], axis=mybir.AxisListType.X)
            ps_t2 = ps_m.tile([1, HL], FP, name="ps_t2", tag="sc")
            nc.tensor.transpose(ps_t2[:], m8[:], ident[0:HL, 0:HL])
            m8T = epool.tile([1, HL], BF16, name="m8T", tag="m8T", bufs=1)
            nc.vector.tensor_copy(m8T[:], ps_t2[:])
            for al in range(HL):
                ps_b = ps_m.tile([128, 1], FP, name="ps_b", tag="sc")
                nc.tensor.matmul(ps_b[:], ones_bf[0:1, :], m8T[0:1, al:al + 1])
                nc.vector.tensor_scalar(bias_sb[:, al:al + 1], ps_b[:],
                                        -0.125, 10.0, MULT, ADD)

        emit_shift_all()

        # ---- per-head B (triangle scores+exp) / mirror / C / norm / F ----
        # Tail work of head h (last C steps, normalization, projection) is
        # deferred into head h+1's B strips via `carry` so the PE never
        # drains while the last mirror DMA / recip / shift latencies play
        # out.
        carry = []

        def emit_head(al):
            g, lo = al // 2, 64 * (al % 2)
            cpart_box = [None]   # allocated lazily at C(0), after carry pops

            def emit_B(i):
                # direct strip i: score tiles (i, j>=i), then exp into Edir
                W = (NB - i) * 128
                for s0 in range(0, W, 1024):
                    w = min(1024, W - s0)
                    sc = ps_m.tile([128, 1024], FP, name="sc", tag="sc")
                    for ns in range(0, w, 512):
                        nw = min(512, w - ns)
                        nc.tensor.matmul(
                            sc[:, ns:ns + nw],
                            pT_sb[lo:lo + 64, g, 128 * i:128 * (i + 1)],
                            pT_sb[lo:lo + 64, g,
                                  128 * i + s0 + ns:128 * i + s0 + ns + nw],
                            tile_position=(lo, 0))
                    k0 = base[i] + s0 // 128
                    nc.scalar.activation(
                        Edir[:, k0:k0 + w // 128, :].rearrange("p a b -> p (a b)"),
                        sc[:, 0:w], Exp, scale=1.0 / 8.0,
                        bias=bias_sb[:, al:al + 1])

            def emit_mirror(i):
                # lower tiles (i, j>i) -> Mbuf slot i via DMA XBAR transpose
                n = NB - 1 - i
                if n == 0:
                    return
                dst = MbufP[:, al % 2, i, 0:n, :] if i < 2 \
                    else Mbuf[:, i, 0:n, :]
                nc.sync.dma_start_transpose(
                    dst,
                    Edir[:, base[i] + 1:base[i] + 1 + n, :]
                    .rearrange("p a b -> p (a b)"))

            def emit_C(i):
                # cpart[:, s] += p65_i^T @ E[strip i, all s]; row 64 = sums
                if cpart_box[0] is None:
                    cpart_box[0] = ps_m.tile([dk + 1, S], FP, name="cpart",
                                             tag="cp", bufs=1)
                cpart = cpart_box[0]
                lhsT = p65[:, i, al, :]
                for b in range(4):
                    # mirror tiles (m, i) for m in this bank: m<2 from the
                    # parity buffer, m>=2 from Mbuf; then direct tiles
                    pieces = []
                    m0, m1 = 4 * b, min(4 * b + 4, i)
                    pp = min(m1, 2)
                    if m0 < pp:
                        st = (al % 2) * 2 * (NB - 1) + 14 * m0 + i - 1
                        pieces.append((m0, pp,
                                       MPflat[:, st:st + 14 * (pp - m0 - 1)
                                              + 1:14, :]))
                    fm0 = max(m0, 2)
                    if fm0 < m1:
                        st = 14 * fm0 + i - 1
                        pieces.append((fm0, m1,
                                       Mflat[:, st:st + 14 * (m1 - fm0 - 1)
                                             + 1:14, :]))
                    t0 = max(4 * b, i)
                    if t0 < 4 * b + 4:
                        pieces.append((t0, 4 * b + 4,
                                       Edir[:, base[i] + t0 - i:
                                            base[i] + 4 * b + 4 - i, :]))
                    for k, (s0, s1, rhs) in enumerate(pieces):
                        nc.tensor.matmul(
                            cpart[:, 128 * s0:128 * s1], lhsT, rhs,
                            start=(i == 0 and k == 0),
                            stop=(i == NB - 1 and k == len(pieces) - 1),
                            skip_group_check=True)

            # Mirrors lag two strips so the carried tail C-steps of the
            # PREVIOUS head (popped below, reading the old Mbuf slots) are
            # emitted before this head's mirror writes reuse those slots.
            LAG = 3
            for i in range(NB):
                emit_B(i)
                if carry:
                    carry.pop(0)()
                if "M" in phases:
                    emit_mirror(i)
                if "C" in phases and 0 <= i - LAG <= NB - 1 - LAG:
                    emit_C(i - LAG)

            if "C" not in phases:
                return
            # tail of this head, interleaved into the next head's B strips
            cpart_sb = npool.tile([dk + 1, S], FP, name="cpart_sb",
                                  tag="cpart_sb", bufs=1)
            recip = npool.tile([1, S], MM, name="recip", tag="recip", bufs=1)
            normD = npool.tile([128, S], BF16, name="normD", tag="normD",
                                bufs=1)

            def t_c14():
                emit_C(NB - 2)

            def t_c15():
                emit_C(NB - 1)
                # free cpart early (Act drain): the sole PSUM reader, so the
                # next head's cpart allocation only waits on this one copy
                nc.scalar.copy(cpart_sb[:], cpart_box[0][:])

            def t_recip():
                with nc.allow_low_precision(reason="recip feeds fp32r bc"):
                    nc.vector.reciprocal(recip[:], cpart_sb[dk:dk + 1, :])

            def t_norm(hh):
                bc = ps_m.tile([128, 1024], FP, name="bc", tag="sc")
                for ns in range(2):
                    nc.tensor.matmul(
                        bc[0:dk, 512 * ns:512 * (ns + 1)],
                        ones_r[0:1, :],
                        recip[0:1, 1024 * hh + 512 * ns:
                              1024 * hh + 512 * (ns + 1)])
                bc_sb = npool.tile([dk, 1024], FP, name="bc_sb", tag="bc_sb",
                                   bufs=1)
                nc.vector.tensor_copy(bc_sb[:], bc[0:dk, :])
                nc.vector.tensor_tensor(
                    normD[0:dk, 1024 * hh:1024 * (hh + 1)],
                    cpart_sb[0:dk, 1024 * hh:1024 * (hh + 1)], bc_sb[:], MULT)
                if hh == 1:
                    # upper half = lower shifted left by one (odd c's)
                    nc.sync.dma_start(normD[64:128, 0:S - 1],
                                      normD[0:64, 1:S])

            def t_proj():
                fps = ps_m.tile([128, 1024], FP, name="fps", tag="sc")
                for ns in range(2):
                    nc.tensor.matmul(fps[:, 512 * ns:512 * (ns + 1)],
                                     ones_bf[0:1, 0:128],
                                     bo_bf[0:1, 512 * ns:512 * (ns + 1)],
                                     start=True, stop=False,
                                     skip_group_check=True)
                for tp in range(TP):
                    for ns in range(2):
                        nc.tensor.matmul(
                            fps[:, 512 * ns:512 * (ns + 1)],
                            normD[:, 2 * tp::NB],
                            woT_sb[:, tp, 512 * ns:512 * (ns + 1)],
                            start=False, stop=(tp == TP - 1),
                            skip_group_check=True)
                fsb = npool.tile([128, D], FP, name="fsb", tag="fsb", bufs=1)
                nc.vector.tensor_copy(fsb[:], fps[:])
                nc.sync.dma_start(out_d[128 * al:128 * (al + 1), :], fsb[:])

            if "N" in phases:
                carry.extend([lambda: emit_C(NB - 3), t_c14, t_c15, t_recip,
                              lambda: t_norm(0), lambda: t_norm(1), t_proj])
            else:
                carry.extend([lambda: emit_C(NB - 3), t_c14, t_c15])

        for al in range(HL):
            emit_head(al)
        while carry:
            carry.pop(0)()

        if loop_cm is not None:
            loop_cm.__exit__(None, None, None)

    return nc


def _split_excess_waits(nc, max_waits=1):
    """This toolchain's walrus accepts only one sync-wait per instruction;
    hoist extra waits onto NoOps inserted just before."""
    fn = nc.m.functions[0]
    n_new = 0
    for blk in fn.blocks:
        new_insts = []
        for inst in blk.instructions:
            si = getattr(inst, 'sync_info', None)
            if si is not None and si.on_wait is not None \
                    and len(si.on_wait) > max_waits:
                waits = list(si.on_wait)
                while len(waits) > max_waits:
                    chunk, waits = waits[:max_waits], waits[max_waits:]
                    n_new += 1
                    new_insts.append(mybir.InstNoOp(
                        name=f"I-waitsplit-{n_new}", engine=inst.engine,
                        ins=[], outs=[],
                        sync_info=mybir.SyncInfo(on_wait=chunk, on_update=[]),
                        bass_nofuse=True))
                si.on_wait = waits
            new_insts.append(inst)
        blk.instructions = new_insts
    return n_new


class _PjrtRunner:
    def __init__(self, nc, n_cores):
        import jax
        from jax.sharding import Mesh, PartitionSpec
        from jax.experimental.shard_map import shard_map
        from concourse.bass2jax import (_bass_exec_p, partition_id_tensor,
                                        install_neuronx_cc_hook)
        install_neuronx_cc_hook()
        self.jax = jax
        self.n_cores = n_cores
        pname = nc.partition_id_tensor.name if nc.partition_id_tensor else None
        in_names, out_names, out_avals, zero_outs = [], [], [], []
        for alloc in nc.m.functions[0].allocations:
            if not isinstance(alloc, mybir.MemoryLocationSet):
                continue
            name = alloc.memorylocations[0].name
            if alloc.kind == "ExternalInput":
                if name != pname:
                    in_names.append(name)
            elif alloc.kind == "ExternalOutput":
                shape = tuple(alloc.tensor_shape)
                dtype = mybir.dt.np(alloc.dtype)
                out_names.append(name)
                out_avals.append(jax.core.ShapedArray(shape, dtype))
                zero_outs.append(np.zeros(shape, dtype))
        self.in_names, self.out_names = in_names, out_names
        self.out_avals, self.zero_outs = out_avals, zero_outs
        n_params, n_outs = len(in_names), len(out_avals)
        self.n_params = n_params
        all_in = in_names + out_names + ([pname] if pname else [])

        def _body(*args):
            operands = list(args)
            if pname is not None:
                operands.append(partition_id_tensor())
            return tuple(_bass_exec_p.bind(
                *operands, out_avals=tuple(out_avals), in_names=tuple(all_in),
                out_names=tuple(out_names), lowering_input_output_aliases=(),
                sim_require_finite=True, sim_require_nnan=True, nc=nc))

        devices = jax.devices()[:n_cores]
        self.mesh = Mesh(np.asarray(devices), ("core",))
        in_specs = (PartitionSpec("core"),) * (n_params + n_outs)
        out_specs = (PartitionSpec("core"),) * n_outs
        self.fn = jax.jit(
            shard_map(_body, mesh=self.mesh, in_specs=in_specs,
                      out_specs=out_specs, check_rep=False), keep_unused=True)
        self.PartitionSpec = PartitionSpec

    def run(self, in_maps):
        jax = self.jax
        per_core = [[np.asarray(m[n]) for n in self.in_names] for m in in_maps]
        concat_in = [np.concatenate([per_core[c][i] for c in range(self.n_cores)],
                                    axis=0) for i in range(self.n_params)]
        concat_zeros = [np.zeros((self.n_cores * z.shape[0], *z.shape[1:]),
                                 z.dtype) for z in self.zero_outs]
        sharding = jax.sharding.NamedSharding(self.mesh, self.PartitionSpec("core"))
        dev_in = [jax.device_put(a, sharding) for a in concat_in + concat_zeros]
        outs = self.fn(*dev_in)
        jax.block_until_ready(outs)
        return [
            {n: np.asarray(outs[i]).reshape(self.n_cores,
                                            *self.out_avals[i].shape)[c]
             for i, n in enumerate(self.out_names)}
            for c in range(self.n_cores)
        ]


_CACHE = {}

B_, S_, D_, H_, DK_ = 4, 2048, 1024, 16, 64
HL_ = H_ // 2          # heads per device
EL_ = HL_ * DK_        # value-projection width per device


def _make_in_maps(x, Wv, bv, Wo, bo):
    import ml_dtypes
    woT = np.ascontiguousarray(Wo.T).astype(ml_dtypes.bfloat16)
    in_maps = []
    for dev in range(8):
        b, hg = dev // 2, dev % 2
        in_maps.append({
            "xT": np.ascontiguousarray(x[b].T).astype(np.float16),
            "wvT": np.ascontiguousarray(
                Wv[EL_ * hg:EL_ * (hg + 1), :].T).astype(np.float16),
            "woT": woT,
            "bv": np.ascontiguousarray(
                bv[EL_ * hg:EL_ * (hg + 1)]).reshape(1, -1).astype(np.float32),
            "bo": np.ascontiguousarray(bo).reshape(1, -1).astype(np.float32),
        })
    return in_maps


def kernel(x, Wv, bv, Wo, bo):
    x, Wv, bv = np.asarray(x), np.asarray(Wv), np.asarray(bv)
    Wo, bo = np.asarray(Wo), np.asarray(bo)
    if "r" not in _CACHE:
        nc = _build_mha_nc(S=S_, D=D_, HL=HL_, dk=DK_)
        _split_excess_waits(nc)
        _CACHE["r"] = _PjrtRunner(nc, 8)
    r = _CACHE["r"]
    res = r.run(_make_in_maps(x, Wv, bv, Wo, bo))
    out = np.zeros((B_, S_, D_), np.float32)
    for dev in range(8):
        b, hg = dev // 2, dev % 2
        out[b, 1024 * hg:1024 * (hg + 1), :] = res[dev]["out"]
    return out


# revision 40
# speedup vs baseline: 1.2200x; 1.2012x over previous
"""Trainium2 Bass kernel for nn_MultiHeadAttention_910533067646.

Self-contained: builds the Bass module, shards the full inputs across the
8 NeuronCores (data-parallel over batch x tensor-parallel over heads), runs
via PJRT, and reassembles the full output.

The reference module applies one shared projection p = x @ Wv.T + bv for
q=k=v, per-head softmax(p ph.T/8) @ ph, then a head-major (bugged) reshape
and output projection. The bugged reshape maps each head's attention output
to a disjoint 128-row block of the final output, so no cross-device
reduction is needed: device (b, hg) computes output rows
[1024*hg, 1024*hg+1024) of batch b.
"""
import numpy as np

from collections import deque
from contextlib import ExitStack

import concourse.bass as bass
import concourse.mybir as mybir
import concourse.tile as tile
from concourse.masks import make_identity

FP = mybir.dt.float32
FPR = mybir.dt.float32r
FP16 = mybir.dt.float16
BF16 = mybir.dt.bfloat16
Exp = mybir.ActivationFunctionType.Exp
ADD = mybir.AluOpType.add
MULT = mybir.AluOpType.mult


def _build_mha_nc(S=2048, D=1024, HL=8, dk=64, phases="ABCNF", MM=FPR,
                 loop_bcnf=1, dbg=False):
    EL = HL * dk            # local width of the value projection
    KK = D // 128           # contraction k-tiles
    NG = HL // 2            # head pairs
    NB = S // 128           # 128-row blocks of the sequence
    NBH = NB // 2           # blocks per sq-half
    SQH = S // 2            # sq-half width
    TT = D // dk            # total heads (= reshape block count)
    W = min(512, SQH)       # N-slice width for panels
    NSL = SQH // W
    WS = min(512, S)        # N-slice for pT phase
    NSS = S // WS
    WD = min(512, D)        # N-slice over D (output projection)
    NSD = D // WD
    assert EL <= 512 and SQH == D and S == 128 * TT and TT % 2 == 0

    nc = bass.Bass("TRN2")
    xT_d = nc.dram_tensor("xT", [D, S], FP16, kind="ExternalInput")
    wvT_d = nc.dram_tensor("wvT", [D, EL], FP16, kind="ExternalInput")
    woT_d = nc.dram_tensor("woT", [D, D], BF16, kind="ExternalInput")
    bv_d = nc.dram_tensor("bv", [1, EL], FP, kind="ExternalInput")
    bo_d = nc.dram_tensor("bo", [1, D], FP, kind="ExternalInput")
    sel_d = nc.dram_tensor("sel", [2, 128], FP, kind="ExternalInput")
    if dbg:
        dbg_pT = nc.dram_tensor("dbg_pT", [128, NG * S], FP, kind="ExternalOutput")
        dbg_p = nc.dram_tensor("dbg_p", [128, NB * EL], FP16, kind="ExternalOutput")
        dbg_sums = nc.dram_tensor("dbg_sums", [128, 2 * NB * 2], FP,
                                  kind="ExternalOutput")
        dbg_recipT = nc.dram_tensor("dbg_recipT", [NB, 2 * 128], FP,
                                    kind="ExternalOutput")
        dbg_norm = nc.dram_tensor("dbg_norm", [128, S], FP, kind="ExternalOutput")
        dbg_rows = nc.dram_tensor("dbg_rows", [2, 2 * SQH], FP, kind="ExternalOutput")
        dbg_bc = nc.dram_tensor("dbg_bc", [128, 2 * SQH], FP, kind="ExternalOutput")
    out_d = nc.dram_tensor("out", [128 * HL, D], FP, kind="ExternalOutput")

    with ExitStack() as stk:
        tc = stk.enter_context(tile.TileContext(nc))
        const = stk.enter_context(tc.tile_pool(name="const", bufs=1))
        ppool = stk.enter_context(tc.tile_pool(name="ppool", bufs=1))
        epool = stk.enter_context(tc.tile_pool(name="epool", bufs=10))
        ps_m = stk.enter_context(tc.tile_pool(name="ps_m", bufs=3, space="PSUM"))

        bv_sb = const.tile([1, EL], MM, name="bv_sb")
        bo_sb = const.tile([1, D], FP, name="bo_sb")
        bo_bf = const.tile([1, D], BF16, name="bo_bf")
        ones32 = const.tile([1, 512], FP, name="ones32")
        ones_sb = const.tile([1, 512], MM, name="ones_sb")
        sel_sb = const.tile([2, 128], MM, name="sel_sb")
        ones_bf = const.tile([1, 128], BF16, name="ones_bf")
        ident = const.tile([128, 128], FP, name="ident")
        ident16 = const.tile([128, 128], FP16, name="ident16")
        ones_h = const.tile([1, 512], FP16, name="ones_h")
        bv_h = const.tile([1, EL], FP16, name="bv_h")
        bias_sb = const.tile([128, HL], FP, name="bias_sb")
        nc.sync.dma_start(bv_sb[:], bv_d[:].bitcast(MM))
        nc.sync.dma_start(bo_sb[:], bo_d[:])
        nc.gpsimd.memset(ones32[:], 1.0)
        nc.vector.tensor_copy(ones_sb[:], ones32[:])
        nc.vector.tensor_copy(ones_bf[:], ones32[0:1, 0:128])
        nc.sync.dma_start(sel_sb[:], sel_d[:].bitcast(MM))
        make_identity(nc, ident[:])
        make_identity(nc, ident16[:])
        nc.gpsimd.memset(ones_h[:], 1.0)
        nc.vector.tensor_copy(bv_h[:], bv_sb[:])
        nc.vector.tensor_copy(bo_bf[:], bo_sb[:])

        pT_sb = ppool.tile([128, NG, S], FP16, name="pT_sb")
        p_sb = ppool.tile([128, NB, EL], FP16, name="p_sb")

        # woT (bf16) fits alongside the xT staging pool: load it during
        # phase A so its DMA overlaps the projection compute
        wpool = stk.enter_context(tc.tile_pool(name="wpool", bufs=1))
        woT_dup = wpool.tile([128, TT, D], BF16, name="woT_dup")

        xt_ctx = tc.tile_pool(name="xtpool", bufs=1)
        xtpool = xt_ctx.__enter__()
        wvT_sb = xtpool.tile([128, KK, EL], FP16, name="wvT_sb")
        xT_sb = xtpool.tile([128, KK, S], FP16, name="xT_sb")
        nc.sync.dma_start(wvT_sb[:],
                          wvT_d[:].rearrange("(kk p) e -> p kk e", p=128))
        for kk in range(KK):
            nc.sync.dma_start(xT_sb[:, kk, :], xT_d[128 * kk:128 * (kk + 1), :])
        wsrc = woT_d[:].rearrange("(t p) e -> p t e", p=dk)
        nc.sync.dma_start(woT_dup[0:dk, :, :], wsrc)
        nc.sync.dma_start(woT_dup[dk:2 * dk, :, :], wsrc)

        # ---- projection work units (phase A), emitted interleaved ----
        def emit_pT(g, ns):
            ps = ps_m.tile([128, WS], FP, name="ps_pt", tag="scores")
            for kk in range(KK):
                nc.tensor.matmul(ps[:], wvT_sb[:, kk, 128 * g:128 * (g + 1)],
                                 xT_sb[:, kk, WS * ns:WS * (ns + 1)],
                                 start=(kk == 0), stop=False)
            nc.tensor.matmul(ps[:], bv_h[0:1, 128 * g:128 * (g + 1)],
                             ones_h[0:1, 0:WS], start=False, stop=True)
            nc.vector.tensor_copy(pT_sb[:, g, WS * ns:WS * (ns + 1)], ps[:])

        def emit_p(j):
            ps = ps_m.tile([128, EL], FP, name="ps_p", tag="scores")
            for kk in range(KK):
                nc.tensor.matmul(ps[:], xT_sb[:, kk, 128 * j:128 * (j + 1)],
                                 wvT_sb[:, kk, :], start=(kk == 0), stop=False)
            nc.tensor.matmul(ps[:], ones_h[0:1, 0:128], bv_h[0:1, :],
                             start=False, stop=True)
            nc.vector.tensor_copy(p_sb[:, j, :], ps[:])

        def emit_T(g, j):
            ps = ps_m.tile([128, 128], FP16, name="ps_t0", tag="scores")
            nc.tensor.transpose(ps[:], p_sb[:, j, 128 * g:128 * (g + 1)],
                                ident16[:])
            nc.vector.tensor_copy(pT_sb[:, g, 128 * j:128 * (j + 1)], ps[:])

        proj_q = deque()
        for g in range(1, NG):
            for j in range(NB):
                proj_q.append(("T", g, j))

        def emit_proj(n):
            while n > 0 and proj_q:
                u = proj_q.popleft()
                if u[0] == "p":
                    emit_p(u[1])
                elif u[0] == "T":
                    emit_T(u[1], u[2])
                else:
                    emit_pT(u[1], u[2])
                n -= 1

        # prefix: pT for head-pair 0 and all of p (B/C/shift need them)
        for ns in range(NSS):
            emit_pT(0, ns)
        for j in range(NB):
            emit_p(j)

        post_pools = {}

        def ensure_post_pools():
            # opened once phase A is fully emitted: reuses xT address space
            if post_pools:
                return
            xt_ctx.__exit__(None, None, None)
            post_pools["n"] = stk.enter_context(tc.tile_pool(name="npool", bufs=2))
            post_pools["b"] = stk.enter_context(tc.tile_pool(name="bpool", bufs=2))
            post_pools["f"] = stk.enter_context(tc.tile_pool(name="fpool", bufs=2))
            post_pools["r"] = stk.enter_context(tc.tile_pool(name="rpool", bufs=2))
            post_pools["woT"] = woT_dup

        if "B" not in phases:
            emit_proj(len(proj_q))
        ensure_post_pools()   # transposes don't need xtpool; open early
        def emit_shift_all():
            # bias[:, al] = 10 - max_k(|p_k,al|^2)/8 for every local head al.
            # Cauchy-Schwarz: s[k,q] <= max_diag, so exp(s/8 + bias) <= e^10
            # keeps fp16 E from overflowing.
            dg = epool.tile([128, HL, NB], FP, name="dg", tag="dg", bufs=1)
            for j in range(NB):
                scr = epool.tile([128, HL, dk], FP, name="scr", tag="scr", bufs=2)
                nc.vector.tensor_tensor(
                    scr[:].rearrange("p a b -> p (a b)"), p_sb[:, j, :],
                    p_sb[:, j, :], MULT)
                nc.vector.reduce_sum(dg[:, :, j], scr[:],
                                     axis=mybir.AxisListType.X)
            cm8 = epool.tile([128, HL], FP, name="cm8", tag="cm8", bufs=1)
            nc.vector.reduce_max(cm8[:], dg[:], axis=mybir.AxisListType.X)
            ps_t1 = ps_m.tile([HL, 128], FP, name="ps_t1", tag="scores")
            nc.tensor.transpose(ps_t1[:], cm8[:], ident[:])
            cmT = epool.tile([HL, 128], FP, name="cmT", tag="cmT", bufs=1)
            nc.vector.tensor_copy(cmT[:], ps_t1[:])
            m8 = epool.tile([HL, 1], FP, name="m8", tag="m8", bufs=1)
            nc.vector.reduce_max(m8[:], cmT[:], axis=mybir.AxisListType.X)
            ps_t2 = ps_m.tile([1, HL], FP, name="ps_t2", tag="scores")
            nc.tensor.transpose(ps_t2[:], m8[:], ident[0:HL, 0:HL])
            m8T = epool.tile([1, HL], BF16, name="m8T", tag="m8T", bufs=1)
            nc.vector.tensor_copy(m8T[:], ps_t2[:])
            for al in range(HL):
                ps_b = ps_m.tile([128, 1], FP, name="ps_b", tag="scores")
                nc.tensor.matmul(ps_b[:], ones_bf[0:1, :], m8T[0:1, al:al + 1])
                nc.vector.tensor_scalar(bias_sb[:, al:al + 1], ps_b[:],
                                        -0.125, 10.0, MULT, ADD)

        loop_cm = None
        if loop_bcnf > 1:
            emit_proj(len(proj_q))
            ensure_post_pools()
            loop_cm = tc.For_i(0, loop_bcnf, 1)
            loop_cm.__enter__()
        shift_done = [False]
        pending_nf = [None]
        for g in range(NG if "B" in phases else 0):
            if not shift_done[0]:
                emit_shift_all()
                shift_done[0] = True
            sums = epool.tile([128, 2, NB, 2], FP, name="sums", tag="sums", bufs=2)
            outT_sb_box = [None]
            cpart = [None, None]

            W16 = min(512, SQH)
            NS16 = SQH // W16

            IH = NB // 2

            def emit_C_one(h, i, ns, a2, E):
                al = 2 * g + a2
                if cpart[h] is None:
                    cpart[h] = ps_m.tile([128, SQH], FP, name="cp",
                                         tag="cpart", bufs=1)
                nc.tensor.matmul(
                    cpart[h][64 * a2:64 * (a2 + 1), W16 * ns:W16 * (ns + 1)],
                    p_sb[:, i, dk * al:dk * (al + 1)],
                    E[:, W16 * ns:W16 * (ns + 1)],
                    tile_position=(0, 64 * a2),
                    start=(i % IH == 0), stop=(i % IH == IH - 1),
                    skip_group_check=True)

            def drain_C(h, first):
                if outT_sb_box[0] is None:
                    outT_sb_box[0] = post_pools["n"].tile(
                        [128, 2, SQH], FP, name="outT_sb", tag="outT_sb", bufs=2)
                outT_sb = outT_sb_box[0]
                if first:
                    nc.vector.tensor_copy(outT_sb[:, h, :], cpart[h][:])
                else:
                    nc.vector.tensor_tensor(outT_sb[:, h, :], cpart[h][:],
                                            outT_sb[:, h, :], ADD)
                cpart[h] = None

            for h in range(2):
                prev = None
                for i in range(NB):
                    emit_proj(2)
                    if not proj_q and not post_pools:
                        ensure_post_pools()
                    if h == 0 and i == 2 and pending_nf[0] is not None:
                        pending_nf[0]()
                        pending_nf[0] = None
                    cur = []
                    for a2 in range(2):
                        al_ = 2 * g + a2
                        lo, hi = 64 * a2, 64 * (a2 + 1)
                        # C matmuls of this head for step i-1 first: they
                        # depend only on this head's E(i-1), so they fill
                        # the PE gap while the other head's exp still runs
                        if prev is not None and "C" in phases:
                            for k in range(NS16):
                                ns = (k + a2) % NS16
                                emit_C_one(h, i - 1, ns, a2, prev[a2])
                            if a2 == 1 and i == IH:
                                drain_C(h, first=True)
                        sc = ps_m.tile([128, SQH], FP, name="sc", tag="scores")
                        for ns in range(NSL):
                            nc.tensor.matmul(
                                sc[:, W * ns:W * (ns + 1)],
                                pT_sb[lo:hi, g, 128 * i:128 * (i + 1)],
                                pT_sb[lo:hi, g,
                                      SQH * h + W * ns:SQH * h + W * (ns + 1)],
                                tile_position=(64 * a2, 0))
                        E = epool.tile([128, SQH], FP16, name="E", tag="E")
                        if h == 0 or i % 4 == 0:
                            # row sums on DVE: saves the Act accumulator
                            # readout on half the tiles (Act is the bottleneck)
                            nc.scalar.activation(E[:], sc[:], Exp,
                                                 scale=1.0 / 8.0,
                                                 bias=bias_sb[:, al_:al_ + 1])
                            nc.vector.reduce_sum(sums[:, a2, i, h:h + 1], E[:],
                                                 axis=mybir.AxisListType.X)
                        else:
                            nc.scalar.activation(
                                E[:], sc[:], Exp, scale=1.0 / 8.0,
                                bias=bias_sb[:, al_:al_ + 1],
                                accum_out=sums[:, a2, i, h:h + 1])
                        cur.append(E)
                    prev = cur
                if "C" in phases:
                    for k in range(NS16):
                        for a2 in range(2):
                            emit_C_one(h, NB - 1, (k + a2) % NS16, a2, prev[a2])
                    drain_C(h, first=False)

            emit_proj(len(proj_q))  # flush any phase-A leftovers
            ensure_post_pools()
            woT_dup = post_pools["woT"]
            if "N" not in phases:
                continue

            # ---- normalization ----

            def make_nf(g=g, sums=sums, outT_sb_box=outT_sb_box):
                def nf():
                    if dbg and g == 0:
                        nc.sync.dma_start(dbg_pT[:].bitcast(MM),
                                          pT_sb[:].rearrange("p a b -> p (a b)"))
                        nc.sync.dma_start(dbg_p[:], p_sb[:].rearrange("p a b -> p (a b)"))
                        nc.sync.dma_start(dbg_sums[:],
                                          sums[:].rearrange("p a b c -> p (a b c)"))
                    tot = epool.tile([128, 2, NB], FP, name="tot", tag="tot", bufs=2)
                    recipT = post_pools["r"].tile([NB, 2, 128], FP, name="recipT",
                                                  tag="recipT")
                    for a2 in range(2):
                        nc.vector.tensor_tensor(tot[:, a2, :], sums[:, a2, :, 0],
                                                sums[:, a2, :, 1], ADD)
                        nc.vector.reciprocal(tot[:, a2, :], tot[:, a2, :])
                        ps_t = ps_m.tile([NB, 128], FP, name="ps_t", tag="scores")
                        nc.tensor.transpose(ps_t[:], tot[:, a2, :], ident[:])
                        nc.vector.tensor_copy(recipT[:, a2, :], ps_t[:])
                    if dbg and g == 0:
                        nc.sync.dma_start(dbg_recipT[:],
                                          recipT[:].rearrange("p a b -> p (a b)"))
                    norm_g = post_pools["n"].tile([128, S], BF16, name="norm_g", tag="nr")
                    for h in range(2):
                        rows2 = post_pools["r"].tile([2, SQH], MM, name="rows2",
                                                     tag="rows", bufs=2)
                        for a2 in range(2):
                            nc.sync.dma_start(
                                rows2[a2:a2 + 1, :],
                                recipT[NBH * h:NBH * (h + 1), a2, :].bitcast(MM))
                        # bc[p, n] = rows2[0, n] for p<64 else rows2[1, n] (K=2 matmul)
                        bc_ps = ps_m.tile([128, SQH], FP, name="bc_ps", tag="scores")
                        for ns in range(NSL):
                            nc.tensor.matmul(bc_ps[:, W * ns:W * (ns + 1)], sel_sb[:],
                                             rows2[:, W * ns:W * (ns + 1)])
                        bc = post_pools["b"].tile([128, SQH], FP, name="bc", tag="bc")
                        nc.vector.tensor_copy(bc[:], bc_ps[:])
                        if dbg and g == 0:
                            nc.sync.dma_start(dbg_rows[:, SQH * h:SQH * (h + 1)].bitcast(MM),
                                              rows2[:])
                            nc.sync.dma_start(dbg_bc[:, SQH * h:SQH * (h + 1)], bc[:])
                        nc.vector.tensor_tensor(norm_g[:, SQH * h:SQH * (h + 1)],
                                                outT_sb_box[0][:, h, :], bc[:], MULT)

                    if "F" not in phases:
                        return
                    if dbg and g == 0:
                        pass  # dbg_norm dump disabled (norm_g is bf16 now)
                    # ---- output projection (4-quadrant: a2 on rows, h on cols) ----
                    fps = [ps_m.tile([128, D], FP, name="fp_a", tag="scores"),
                           ps_m.tile([128, D], FP, name="fp_b", tag="cpart", bufs=1)]
                    for a2 in range(2):
                        for ns in range(NSD):
                            nc.tensor.matmul(fps[a2][:, WD * ns:WD * (ns + 1)],
                                             ones_bf[0:1, 0:128],
                                             bo_bf[0:1, WD * ns:WD * (ns + 1)],
                                             start=True, stop=False, skip_group_check=True)
                    for ns in range(NSD):
                        for t in range(TT):
                            for a2 in range(2):
                                lo = 64 * a2
                                nc.tensor.matmul(
                                    fps[a2][:, WD * ns:WD * (ns + 1)],
                                    norm_g[lo:lo + 64, t::TT],
                                    woT_dup[lo:lo + 64, t, WD * ns:WD * (ns + 1)],
                                    tile_position=(lo, 0),
                                    start=False, stop=(t == TT - 1),
                                    skip_group_check=True)
                    for a2 in range(2):
                        fsb = post_pools["f"].tile([128, D], FP, name="fsb", tag="fsb")
                        nc.vector.tensor_copy(fsb[:], fps[a2][:])
                        al = 2 * g + a2
                        nc.sync.dma_start(out_d[128 * al:128 * (al + 1), :], fsb[:])

                return nf

            if "N" in phases:
                make_nf()()

        if pending_nf[0] is not None:
            pending_nf[0]()
            pending_nf[0] = None
        if loop_cm is not None:
            loop_cm.__exit__(None, None, None)

    return nc


def _split_excess_waits(nc, max_waits=1):
    """This toolchain's walrus accepts only one sync-wait per instruction;
    hoist extra waits onto NoOps inserted just before."""
    fn = nc.m.functions[0]
    n_new = 0
    for blk in fn.blocks:
        new_insts = []
        for inst in blk.instructions:
            si = getattr(inst, 'sync_info', None)
            if si is not None and si.on_wait is not None \
                    and len(si.on_wait) > max_waits:
                waits = list(si.on_wait)
                while len(waits) > max_waits:
                    chunk, waits = waits[:max_waits], waits[max_waits:]
                    n_new += 1
                    new_insts.append(mybir.InstNoOp(
                        name=f"I-waitsplit-{n_new}", engine=inst.engine,
                        ins=[], outs=[],
                        sync_info=mybir.SyncInfo(on_wait=chunk, on_update=[]),
                        bass_nofuse=True))
                si.on_wait = waits
            new_insts.append(inst)
        blk.instructions = new_insts
    return n_new


class _PjrtRunner:
    def __init__(self, nc, n_cores):
        import jax
        from jax.sharding import Mesh, PartitionSpec
        from jax.experimental.shard_map import shard_map
        from concourse.bass2jax import (_bass_exec_p, partition_id_tensor,
                                        install_neuronx_cc_hook)
        install_neuronx_cc_hook()
        self.jax = jax
        self.n_cores = n_cores
        pname = nc.partition_id_tensor.name if nc.partition_id_tensor else None
        in_names, out_names, out_avals, zero_outs = [], [], [], []
        for alloc in nc.m.functions[0].allocations:
            if not isinstance(alloc, mybir.MemoryLocationSet):
                continue
            name = alloc.memorylocations[0].name
            if alloc.kind == "ExternalInput":
                if name != pname:
                    in_names.append(name)
            elif alloc.kind == "ExternalOutput":
                shape = tuple(alloc.tensor_shape)
                dtype = mybir.dt.np(alloc.dtype)
                out_names.append(name)
                out_avals.append(jax.core.ShapedArray(shape, dtype))
                zero_outs.append(np.zeros(shape, dtype))
        self.in_names, self.out_names = in_names, out_names
        self.out_avals, self.zero_outs = out_avals, zero_outs
        n_params, n_outs = len(in_names), len(out_avals)
        self.n_params = n_params
        all_in = in_names + out_names + ([pname] if pname else [])

        def _body(*args):
            operands = list(args)
            if pname is not None:
                operands.append(partition_id_tensor())
            return tuple(_bass_exec_p.bind(
                *operands, out_avals=tuple(out_avals), in_names=tuple(all_in),
                out_names=tuple(out_names), lowering_input_output_aliases=(),
                sim_require_finite=True, sim_require_nnan=True, nc=nc))

        devices = jax.devices()[:n_cores]
        self.mesh = Mesh(np.asarray(devices), ("core",))
        in_specs = (PartitionSpec("core"),) * (n_params + n_outs)
        out_specs = (PartitionSpec("core"),) * n_outs
        self.fn = jax.jit(
            shard_map(_body, mesh=self.mesh, in_specs=in_specs,
                      out_specs=out_specs, check_rep=False), keep_unused=True)
        self.PartitionSpec = PartitionSpec

    def run(self, in_maps):
        jax = self.jax
        per_core = [[np.asarray(m[n]) for n in self.in_names] for m in in_maps]
        concat_in = [np.concatenate([per_core[c][i] for c in range(self.n_cores)],
                                    axis=0) for i in range(self.n_params)]
        concat_zeros = [np.zeros((self.n_cores * z.shape[0], *z.shape[1:]),
                                 z.dtype) for z in self.zero_outs]
        sharding = jax.sharding.NamedSharding(self.mesh, self.PartitionSpec("core"))
        dev_in = [jax.device_put(a, sharding) for a in concat_in + concat_zeros]
        outs = self.fn(*dev_in)
        jax.block_until_ready(outs)
        return [
            {n: np.asarray(outs[i]).reshape(self.n_cores,
                                            *self.out_avals[i].shape)[c]
             for i, n in enumerate(self.out_names)}
            for c in range(self.n_cores)
        ]


_CACHE = {}

B_, S_, D_, H_, DK_ = 4, 2048, 1024, 16, 64
HL_ = H_ // 2          # heads per device
EL_ = HL_ * DK_        # value-projection width per device
_SEL = np.kron(np.eye(2), np.ones((1, 64))).astype(np.float32)


def _make_in_maps(x, Wv, bv, Wo, bo):
    import ml_dtypes
    woT = np.ascontiguousarray(Wo.T).astype(ml_dtypes.bfloat16)
    in_maps = []
    for dev in range(8):
        b, hg = dev // 2, dev % 2
        in_maps.append({
            "xT": np.ascontiguousarray(x[b].T).astype(np.float16),
            "wvT": np.ascontiguousarray(
                Wv[EL_ * hg:EL_ * (hg + 1), :].T).astype(np.float16),
            "woT": woT,
            "bv": np.ascontiguousarray(
                bv[EL_ * hg:EL_ * (hg + 1)]).reshape(1, -1).astype(np.float32),
            "bo": np.ascontiguousarray(bo).reshape(1, -1).astype(np.float32),
            "sel": _SEL,
        })
    return in_maps


def kernel(x, Wv, bv, Wo, bo):
    x, Wv, bv = np.asarray(x), np.asarray(Wv), np.asarray(bv)
    Wo, bo = np.asarray(Wo), np.asarray(bo)
    if "r" not in _CACHE:
        nc = _build_mha_nc(S=S_, D=D_, HL=HL_, dk=DK_)
        _split_excess_waits(nc)
        _CACHE["r"] = _PjrtRunner(nc, 8)
    r = _CACHE["r"]
    res = r.run(_make_in_maps(x, Wv, bv, Wo, bo))
    out = np.zeros((B_, S_, D_), np.float32)
    for dev in range(8):
        b, hg = dev // 2, dev % 2
        out[b, 1024 * hg:1024 * (hg + 1), :] = res[dev]["out"]
    return out

